# revision 4
# baseline (speedup 1.0000x reference)
"""CategorySpecificLinear on 8 TRN2 NeuronCores.

out[b, t, h] = sum_i x[b, t, i] * W[cat_ids[b], i, h] + bias[cat_ids[b], h]

Data-parallel over the batch, 8 samples per core. The host groups
same-category samples onto the same core (greedy bin-packing of category
groups, max ~4 unique cats/core for typical inputs) and packs per-slot
inputs so the device program is fully static:

- x and W are converted to bf16 on the host (fp32 PSUM accumulation keeps
  the result well inside the 2e-2 tolerance); out is returned as bf16 and
  upconverted on the host. This halves DMA traffic vs fp32.
- W is reloaded into a single SBUF-resident buffer only at category-group
  boundaries, via cond-predicated DMAs (skip_entire_dma) driven by per-slot
  flags. Skipped DMAs still increment semaphores so the static schedule
  stands. W is loaded in 4 kk-chunks so a reload pipelines behind the
  previous sample's matmuls (the kk-outer loop consumes chunk j late).
- Matmuls run kk-outer with all 8 PSUM banks live, so each stationary
  x-tile (LDWEIGHTS) is amortized over 4 matmuls.
"""

import os
import sys

import numpy as np

for _p in (
    "/opt/trn_rl_repo",
    os.path.expanduser("~/.axon_site/_ro/trn_rl_repo"),
):
    if os.path.isdir(_p) and _p not in sys.path:
        sys.path.insert(0, _p)

import ml_dtypes  # noqa: E402

import concourse.bass as bass  # noqa: E402
import concourse.mybir as mybir  # noqa: E402
import concourse.tile as tile  # noqa: E402
from concourse import bacc  # noqa: E402
from concourse.bass_utils import run_bass_kernel_spmd  # noqa: E402

NCORES = 8
B, T, I, H, NCAT = 64, 256, 1024, 2048, 32
S = B // NCORES  # samples per core
KK = I // 128  # contraction chunks of 128
NN = H // 512  # output column chunks of 512 (one PSUM bank each)
MM = T // 128  # token tiles of 128 per sample
NCH = 4  # W dma chunks per reload (KK // NCH kk-slices each)
KPC = KK // NCH  # kk slices per W chunk
F32 = mybir.dt.float32
BF16 = mybir.dt.bfloat16
NPBF16 = ml_dtypes.bfloat16

_cache: dict = {}


def _build(has_bias: bool):
    nc = bacc.Bacc(
        "TRN2", target_bir_lowering=False, debug=False, num_devices=NCORES
    )
    x_in = nc.dram_tensor("xs", [S, 128, KK * T], BF16, kind="ExternalInput")
    W_in = nc.dram_tensor("Ws", [S, KK, 128, H], BF16, kind="ExternalInput")
    fl_in = nc.dram_tensor("flags", [1, S], mybir.dt.int32, kind="ExternalInput")
    reps_in = nc.dram_tensor("reps", [1, 1], mybir.dt.int32, kind="ExternalInput")
    if has_bias:
        b_in = nc.dram_tensor("bs", [S, H], F32, kind="ExternalInput")
    out_dt = BF16
    out_o = nc.dram_tensor("out", [S, T, H], out_dt, kind="ExternalOutput")

    with tile.TileContext(nc) as tc:
        with (
            tc.tile_pool(name="const", bufs=1) as cpool,
            tc.tile_pool(name="wbuf", bufs=1) as wpool,
            tc.tile_pool(name="data", bufs=2) as dpool,
            tc.tile_pool(name="mmps", bufs=8, space="PSUM") as mmpool,
        ):
            fl_sb = cpool.tile([1, S], mybir.dt.int32)
            nc.sync.dma_start(fl_sb[:], fl_in[:])
            reps_sb = cpool.tile([1, 1], mybir.dt.int32)
            nc.sync.dma_start(reps_sb[:], reps_in[:])

            cvs = [
                nc.values_load(
                    fl_sb[0:1, s : s + 1],
                    min_val=0,
                    max_val=1,
                    skip_runtime_bounds_check=True,
                )
                for s in range(S)
            ]
            reps_val = nc.values_load(
                reps_sb[0:1, 0:1],
                min_val=1,
                max_val=1 << 20,
                skip_runtime_bounds_check=True,
            )

            with tc.For_i(0, reps_val, 1):
                for s in range(S):
                    cv = cvs[s]
                    # W chunk loads: only at category-group boundaries.
                    # Chunk j covers kk slices [j*KPC, (j+1)*KPC). Alternate
                    # the two HWDGE queues.
                    wts = []
                    for j in range(NCH):
                        wt = wpool.tile([128, KPC * H], BF16, tag=f"w{j}")
                        src = W_in[s, j * KPC : (j + 1) * KPC].rearrange(
                            "kl p h -> p kl h"
                        )
                        eng = nc.sync if j % 2 == 0 else nc.scalar
                        eng.dma_start(
                            wt[:].rearrange("p (kl h) -> p kl h", h=H),
                            src,
                            cond=cv,
                        )
                        wts.append(wt)
                    if has_bias:
                        bb = wpool.tile([128, H], F32, tag="bb")
                        nc.scalar.dma_start(
                            bb[:], b_in[s : s + 1, :].to_broadcast((128, H)), cond=cv
                        )

                    xt = dpool.tile([128, KK * T], BF16, tag="xt")
                    nc.sync.dma_start(xt[:], x_in[s])

                    pss = [
                        mmpool.tile([128, 512], F32, tag="ps", name=f"ps{q}")
                        for q in range(MM * NN)
                    ]
                    for kk in range(KK):
                        wsrc = wts[kk // KPC]
                        koff = (kk % KPC) * H
                        for m in range(MM):
                            lhsT = xt[:, kk * T + m * 128 : kk * T + (m + 1) * 128]
                            for n in range(NN):
                                nc.tensor.matmul(
                                    pss[m * NN + n][:],
                                    lhsT,
                                    wsrc[:, koff + n * 512 : koff + (n + 1) * 512],
                                    start=(kk == 0),
                                    stop=(kk == KK - 1),
                                )

                    ots = [
                        dpool.tile([128, H], out_dt, tag=f"ot{m}", name=f"ot{m}")
                        for m in range(MM)
                    ]
                    for m in range(MM):
                        for n in range(NN):
                            if has_bias:
                                nc.vector.tensor_add(
                                    ots[m][:, n * 512 : (n + 1) * 512],
                                    pss[m * NN + n][:],
                                    bb[:, n * 512 : (n + 1) * 512],
                                )
                            else:
                                nc.vector.tensor_copy(
                                    ots[m][:, n * 512 : (n + 1) * 512],
                                    pss[m * NN + n][:],
                                )
                    for m in range(MM):
                        nc.sync.dma_start(
                            out_o[s, m * 128 : (m + 1) * 128, :], ots[m][:]
                        )

    nc.compile()
    return nc


def _get_nc(has_bias: bool):
    key = ("nc", has_bias)
    if key not in _cache:
        _cache[key] = _build(has_bias)
    return _cache[key]


def _plan(cat_ids: np.ndarray):
    """Pack samples into NCORES bins of S slots, same-category samples
    adjacent, minimizing category-group pieces per bin. Returns per-core
    (sample index list, reload flags)."""
    groups: dict[int, list[int]] = {}
    for i, c in enumerate(cat_ids.tolist()):
        groups.setdefault(c, []).append(i)
    # largest groups first
    order = sorted(groups.items(), key=lambda kv: -len(kv[1]))
    cap = [S] * NCORES
    pieces: list[list[tuple[int, list[int]]]] = [[] for _ in range(NCORES)]
    for cat, idxs in order:
        rest = list(idxs)
        while rest:
            # bin with most remaining capacity; ties -> fewest pieces
            c = max(range(NCORES), key=lambda b: (cap[b], -len(pieces[b])))
            take = min(len(rest), cap[c])
            assert take > 0
            pieces[c].append((cat, rest[:take]))
            cap[c] -= take
            rest = rest[take:]
    idx = np.empty((NCORES, S), dtype=np.int64)
    flags = np.zeros((NCORES, S), dtype=np.int32)
    cats = np.empty((NCORES, S), dtype=np.int32)
    for c in range(NCORES):
        s = 0
        for cat, idxs in pieces[c]:
            flags[c, s] = 1
            for i in idxs:
                idx[c, s] = i
                cats[c, s] = cat
                s += 1
        assert s == S
    return idx, flags, cats


def _make_in_maps(x, cat_ids, W, b, has_bias, plan, reps=1):
    idx, flags, cats = plan
    Wb = np.ascontiguousarray(
        W.reshape(NCAT, KK, 128, H).astype(NPBF16, copy=False)
    )
    in_maps = []
    for c in range(NCORES):
        xs = (
            x[idx[c]]
            .transpose(0, 2, 1)
            .reshape(S, KK, 128, T)
            .transpose(0, 2, 1, 3)
            .reshape(S, 128, KK * T)
            .astype(NPBF16)
        )
        Ws = np.zeros((S, KK, 128, H), dtype=NPBF16)
        for s in range(S):
            if flags[c, s]:
                Ws[s] = Wb[cats[c, s]]
        m = {
            "xs": np.ascontiguousarray(xs),
            "Ws": Ws,
            "flags": flags[c].reshape(1, S),
            "reps": np.full((1, 1), reps, dtype=np.int32),
        }
        if has_bias:
            m["bs"] = np.ascontiguousarray(b[cats[c]].astype(np.float32))
        in_maps.append(m)
    return in_maps


def kernel(x, cat_ids, W, b):
    x = np.ascontiguousarray(np.asarray(x, dtype=np.float32))
    cat_ids = np.asarray(cat_ids, dtype=np.int32)
    W = np.ascontiguousarray(np.asarray(W, dtype=np.float32))
    b = np.ascontiguousarray(np.asarray(b, dtype=np.float32))
    assert x.shape == (B, T, I) and cat_ids.shape == (B,)
    assert W.shape == (NCAT, I, H) and b.shape == (NCAT, H)

    has_bias = bool(np.any(b))
    nc = _get_nc(has_bias)
    plan = _plan(cat_ids)
    in_maps = _make_in_maps(x, cat_ids, W, b, has_bias, plan)

    res = run_bass_kernel_spmd(nc, in_maps, list(range(NCORES)))

    idx = plan[0]
    out = np.empty((B, T, H), dtype=np.float32)
    for c in range(NCORES):
        out[idx[c]] = np.asarray(res.results[c]["out"]).astype(np.float32)
    return out


# revision 8
# speedup vs baseline: 2.0297x; 2.0297x over previous
"""CategorySpecificLinear on 8 TRN2 NeuronCores.

out[b, t, h] = sum_i x[b, t, i] * W[cat_ids[b], i, h] + bias[cat_ids[b], h]

Data-parallel over the batch, 8 samples per core. The host groups
same-category samples onto the same core (greedy bin-packing of category
groups, max ~4 unique cats/core for typical inputs) and packs per-slot
inputs so the device program is fully static:

- x and W are converted to bf16 on the host (fp32 PSUM accumulation keeps
  the result well inside the 2e-2 tolerance); out is returned as bf16 and
  upconverted on the host. This halves DMA traffic vs fp32.
- W is reloaded into a single SBUF-resident buffer only at category-group
  boundaries, via cond-predicated DMAs (skip_entire_dma) driven by per-slot
  flags. Skipped DMAs still increment semaphores so the static schedule
  stands. W is loaded in 4 kk-chunks so a reload pipelines behind the
  previous sample's matmuls (the kk-outer loop consumes chunk j late).
- Matmuls run kk-outer with all 8 PSUM banks live, so each stationary
  x-tile (LDWEIGHTS) is amortized over 4 matmuls.
"""

import contextlib
import os
import sys

import numpy as np

for _p in (
    "/opt/trn_rl_repo",
    os.path.expanduser("~/.axon_site/_ro/trn_rl_repo"),
):
    if os.path.isdir(_p) and _p not in sys.path:
        sys.path.insert(0, _p)

import ml_dtypes  # noqa: E402

import concourse.bass as bass  # noqa: E402
import concourse.mybir as mybir  # noqa: E402
import concourse.tile as tile  # noqa: E402
from concourse import bacc  # noqa: E402
from concourse.bass_utils import run_bass_kernel_spmd  # noqa: E402

NCORES = 8
B, T, I, H, NCAT = 64, 256, 1024, 2048, 32
S = B // NCORES  # samples per core
KK = I // 128  # contraction chunks of 128
NN = H // 512  # output column chunks of 512 (one PSUM bank each)
MM = T // 128  # token tiles of 128 per sample
NCH = 4  # W dma chunks per reload (KK // NCH kk-slices each)
KPC = KK // NCH  # kk slices per W chunk
F32 = mybir.dt.float32
BF16 = mybir.dt.bfloat16
NPBF16 = ml_dtypes.bfloat16

_cache: dict = {}


def _build(has_bias: bool):
    nc = bacc.Bacc(
        "TRN2", target_bir_lowering=False, debug=False, num_devices=NCORES
    )
    x_in = nc.dram_tensor("xs", [S, 128, KK * T], BF16, kind="ExternalInput")
    W_in = nc.dram_tensor("Ws", [S, KK, 128, H], BF16, kind="ExternalInput")
    fl_in = nc.dram_tensor("flags", [1, S], mybir.dt.int32, kind="ExternalInput")
    reps_in = nc.dram_tensor("reps", [1, 1], mybir.dt.int32, kind="ExternalInput")
    if has_bias:
        b_in = nc.dram_tensor("bs", [S, H], F32, kind="ExternalInput")
    out_dt = BF16
    out_o = nc.dram_tensor("out", [S, T, H], out_dt, kind="ExternalOutput")

    with tile.TileContext(nc) as tc, contextlib.ExitStack() as stk:
        with (
            tc.tile_pool(name="const", bufs=1) as cpool,
            tc.tile_pool(name="data", bufs=2) as dpool,
            tc.tile_pool(name="mmps", bufs=8, space="PSUM") as mmpool,
        ):
            # Raw (pool-free) SBUF tensors for the category-resident W buffer
            # and bias: every slot's cond-DMA targets the same tensor, so a
            # skipped reload legitimately reads the previous contents.
            wts = [
                stk.enter_context(nc.sbuf_tensor(f"wt{j}", [128, KPC * H], BF16))
                for j in range(NCH)
            ]
            if has_bias:
                bb = stk.enter_context(nc.sbuf_tensor("bbuf", [128, H], F32))
            fl_sb = cpool.tile([1, S], mybir.dt.int32)
            nc.sync.dma_start(fl_sb[:], fl_in[:])
            reps_sb = cpool.tile([1, 1], mybir.dt.int32)
            nc.sync.dma_start(reps_sb[:], reps_in[:])

            cvs = [
                nc.values_load(
                    fl_sb[0:1, s : s + 1],
                    min_val=0,
                    max_val=1,
                    skip_runtime_bounds_check=True,
                )
                for s in range(S)
            ]
            reps_val = nc.values_load(
                reps_sb[0:1, 0:1],
                min_val=1,
                max_val=1 << 20,
                skip_runtime_bounds_check=True,
            )

            with tc.For_i(0, reps_val, 1):
                for s in range(S):
                    cv = cvs[s]
                    # W chunk loads: only at category-group boundaries.
                    # Chunk j covers kk slices [j*KPC, (j+1)*KPC). Alternate
                    # the two HWDGE queues.
                    for j in range(NCH):
                        src = W_in[s, j * KPC : (j + 1) * KPC].rearrange(
                            "kl p h -> p kl h"
                        )
                        eng = nc.sync if j % 2 == 0 else nc.scalar
                        eng.dma_start(
                            wts[j][:].rearrange("p (kl h) -> p kl h", h=H),
                            src,
                            cond=cv,
                        )
                    if has_bias:
                        nc.scalar.dma_start(
                            bb[:], b_in[s : s + 1, :].to_broadcast((128, H)), cond=cv
                        )

                    xt = dpool.tile([128, KK * T], BF16, tag="xt")
                    nc.sync.dma_start(xt[:], x_in[s])

                    pss = [
                        mmpool.tile([128, 512], F32, tag="ps", name=f"ps{q}")
                        for q in range(MM * NN)
                    ]
                    for kk in range(KK):
                        wsrc = wts[kk // KPC]
                        koff = (kk % KPC) * H
                        for m in range(MM):
                            lhsT = xt[:, kk * T + m * 128 : kk * T + (m + 1) * 128]
                            for n in range(NN):
                                nc.tensor.matmul(
                                    pss[m * NN + n][:],
                                    lhsT,
                                    wsrc[:, koff + n * 512 : koff + (n + 1) * 512],
                                    start=(kk == 0),
                                    stop=(kk == KK - 1),
                                )

                    ots = [
                        dpool.tile([128, H], out_dt, tag=f"ot{m}", name=f"ot{m}")
                        for m in range(MM)
                    ]
                    for m in range(MM):
                        for n in range(NN):
                            if has_bias:
                                nc.vector.tensor_add(
                                    ots[m][:, n * 512 : (n + 1) * 512],
                                    pss[m * NN + n][:],
                                    bb[:, n * 512 : (n + 1) * 512],
                                )
                            else:
                                nc.vector.tensor_copy(
                                    ots[m][:, n * 512 : (n + 1) * 512],
                                    pss[m * NN + n][:],
                                )
                    for m in range(MM):
                        nc.sync.dma_start(
                            out_o[s, m * 128 : (m + 1) * 128, :], ots[m][:]
                        )

    nc.compile()
    return nc


def _get_nc(has_bias: bool):
    key = ("nc", has_bias)
    if key not in _cache:
        _cache[key] = _build(has_bias)
    return _cache[key]


def _plan(cat_ids: np.ndarray):
    """Pack samples into NCORES bins of S slots, same-category samples
    adjacent, minimizing category-group pieces per bin. Returns per-core
    (sample index list, reload flags)."""
    groups: dict[int, list[int]] = {}
    for i, c in enumerate(cat_ids.tolist()):
        groups.setdefault(c, []).append(i)
    # largest groups first
    order = sorted(groups.items(), key=lambda kv: -len(kv[1]))
    cap = [S] * NCORES
    pieces: list[list[tuple[int, list[int]]]] = [[] for _ in range(NCORES)]
    for cat, idxs in order:
        rest = list(idxs)
        while rest:
            # bin with most remaining capacity; ties -> fewest pieces
            c = max(range(NCORES), key=lambda b: (cap[b], -len(pieces[b])))
            take = min(len(rest), cap[c])
            assert take > 0
            pieces[c].append((cat, rest[:take]))
            cap[c] -= take
            rest = rest[take:]
    idx = np.empty((NCORES, S), dtype=np.int64)
    flags = np.zeros((NCORES, S), dtype=np.int32)
    cats = np.empty((NCORES, S), dtype=np.int32)
    for c in range(NCORES):
        s = 0
        for cat, idxs in pieces[c]:
            flags[c, s] = 1
            for i in idxs:
                idx[c, s] = i
                cats[c, s] = cat
                s += 1
        assert s == S
    return idx, flags, cats


def _make_in_maps(x, cat_ids, W, b, has_bias, plan, reps=1):
    idx, flags, cats = plan
    Wb = np.ascontiguousarray(
        W.reshape(NCAT, KK, 128, H).astype(NPBF16, copy=False)
    )
    in_maps = []
    for c in range(NCORES):
        xs = (
            x[idx[c]]
            .transpose(0, 2, 1)
            .reshape(S, KK, 128, T)
            .transpose(0, 2, 1, 3)
            .reshape(S, 128, KK * T)
            .astype(NPBF16)
        )
        Ws = np.zeros((S, KK, 128, H), dtype=NPBF16)
        for s in range(S):
            if flags[c, s]:
                Ws[s] = Wb[cats[c, s]]
        m = {
            "xs": np.ascontiguousarray(xs),
            "Ws": Ws,
            "flags": flags[c].reshape(1, S),
            "reps": np.full((1, 1), reps, dtype=np.int32),
        }
        if has_bias:
            m["bs"] = np.ascontiguousarray(b[cats[c]].astype(np.float32))
        in_maps.append(m)
    return in_maps


def kernel(x, cat_ids, W, b):
    x = np.ascontiguousarray(np.asarray(x, dtype=np.float32))
    cat_ids = np.asarray(cat_ids, dtype=np.int32)
    W = np.ascontiguousarray(np.asarray(W, dtype=np.float32))
    b = np.ascontiguousarray(np.asarray(b, dtype=np.float32))
    assert x.shape == (B, T, I) and cat_ids.shape == (B,)
    assert W.shape == (NCAT, I, H) and b.shape == (NCAT, H)

    has_bias = bool(np.any(b))
    nc = _get_nc(has_bias)
    plan = _plan(cat_ids)
    in_maps = _make_in_maps(x, cat_ids, W, b, has_bias, plan)

    res = run_bass_kernel_spmd(nc, in_maps, list(range(NCORES)))

    idx = plan[0]
    out = np.empty((B, T, H), dtype=np.float32)
    for c in range(NCORES):
        out[idx[c]] = np.asarray(res.results[c]["out"]).astype(np.float32)
    return out


# revision 13
# speedup vs baseline: 6.9593x; 3.4287x over previous
"""CategorySpecificLinear on 8 TRN2 NeuronCores.

out[b, t, h] = sum_i x[b, t, i] * W[cat_ids[b], i, h] + bias[cat_ids[b], h]

Data-parallel over the batch, 8 samples per core. The host groups
same-category samples onto the same core (greedy bin-packing of category
groups, max ~4 unique cats/core for typical inputs) and packs per-slot
inputs so the device program is fully static:

- x and W are converted to bf16 on the host (fp32 PSUM accumulation keeps
  the result well inside the 2e-2 tolerance); out is returned as bf16 and
  upconverted on the host. This halves DMA traffic vs fp32.
- W is reloaded into a single SBUF-resident buffer only at category-group
  boundaries, via cond-predicated DMAs (skip_entire_dma) driven by per-slot
  flags. Skipped DMAs still increment semaphores so the static schedule
  stands. W is loaded in 4 kk-chunks so a reload pipelines behind the
  previous sample's matmuls (the kk-outer loop consumes chunk j late).
- Matmuls run kk-outer with all 8 PSUM banks live, so each stationary
  x-tile (LDWEIGHTS) is amortized over 4 matmuls.
"""

import contextlib
import os
import sys

import numpy as np

for _p in (
    "/opt/trn_rl_repo",
    os.path.expanduser("~/.axon_site/_ro/trn_rl_repo"),
):
    if os.path.isdir(_p) and _p not in sys.path:
        sys.path.insert(0, _p)

import ml_dtypes  # noqa: E402

import concourse.bass as bass  # noqa: E402
import concourse.mybir as mybir  # noqa: E402
import concourse.tile as tile  # noqa: E402
from concourse import bacc  # noqa: E402
from concourse.bass_utils import run_bass_kernel_spmd  # noqa: E402

NCORES = 8
B, T, I, H, NCAT = 64, 256, 1024, 2048, 32
S = B // NCORES  # samples per core
KK = I // 128  # contraction chunks of 128
NN = H // 512  # output column chunks of 512 (one PSUM bank each)
MM = T // 128  # token tiles of 128 per sample
NCH = 4  # W dma chunks per reload (KK // NCH kk-slices each)
KPC = KK // NCH  # kk slices per W chunk
F32 = mybir.dt.float32
BF16 = mybir.dt.bfloat16
NPBF16 = ml_dtypes.bfloat16

_cache: dict = {}


def _build(has_bias: bool, dedupe: bool = True, v2: bool = False):
    V2 = v2
    nc = bacc.Bacc(
        "TRN2", target_bir_lowering=False, debug=False, num_devices=NCORES
    )
    x_in = nc.dram_tensor("xs", [S, 128, KK * T], BF16, kind="ExternalInput")
    W_in = nc.dram_tensor("Ws", [S, KK, 128, H], BF16, kind="ExternalInput")
    fl_in = nc.dram_tensor("flags", [1, S], mybir.dt.int32, kind="ExternalInput")
    reps_in = nc.dram_tensor("reps", [1, 1], mybir.dt.int32, kind="ExternalInput")
    if has_bias:
        b_in = nc.dram_tensor("bs", [S, H], F32, kind="ExternalInput")
    out_dt = BF16
    out_o = nc.dram_tensor("out", [S, T, H], out_dt, kind="ExternalOutput")

    with tile.TileContext(nc) as tc, contextlib.ExitStack() as stk:
        with (
            tc.tile_pool(name="const", bufs=1) as cpool,
            tc.tile_pool(name="data", bufs=2) as dpool,
            tc.tile_pool(name="mmps", bufs=8, space="PSUM") as mmpool,
        ):
            # Raw (pool-free) SBUF tensors for the category-resident W buffer
            # and bias: every slot's cond-DMA targets the same tensor, so a
            # skipped reload legitimately reads the previous contents.
            wts = [
                stk.enter_context(nc.sbuf_tensor(f"wt{j}", [128, KPC * H], BF16))
                for j in range(NCH)
            ]
            if has_bias:
                bb = stk.enter_context(nc.sbuf_tensor("bbuf", [128, H], F32))
            fl_sb = cpool.tile([1, S], mybir.dt.int32)
            nc.sync.dma_start(fl_sb[:], fl_in[:])
            reps_sb = cpool.tile([1, 1], mybir.dt.int32)
            nc.sync.dma_start(reps_sb[:], reps_in[:])

            cvs = [
                nc.values_load(
                    fl_sb[0:1, s : s + 1],
                    min_val=0,
                    max_val=1,
                    skip_runtime_bounds_check=True,
                )
                for s in range(S)
            ]
            reps_val = nc.values_load(
                reps_sb[0:1, 0:1],
                min_val=0,
                max_val=1 << 20,
                skip_runtime_bounds_check=True,
            )

            def _body():
                for s in range(S):
                    cv = cvs[s]
                    # W chunk loads: only at category-group boundaries.
                    # Chunk j covers kk slices [j*KPC, (j+1)*KPC). Alternate
                    # the two HWDGE queues.
                    for j in range(NCH):
                        src = W_in[s, j * KPC : (j + 1) * KPC].rearrange(
                            "kl p h -> p kl h"
                        )
                        eng = nc.sync if j % 2 == 0 else nc.scalar
                        if dedupe:
                            eng.dma_start(
                                wts[j][:].rearrange("p (kl h) -> p kl h", h=H),
                                src,
                                cond=cv,
                            )
                        else:
                            eng.dma_start(
                                wts[j][:].rearrange("p (kl h) -> p kl h", h=H),
                                src,
                            )
                    if has_bias:
                        if dedupe:
                            nc.scalar.dma_start(
                                bb[:],
                                b_in[s : s + 1, :].to_broadcast((128, H)),
                                cond=cv,
                            )
                        else:
                            nc.scalar.dma_start(
                                bb[:], b_in[s : s + 1, :].to_broadcast((128, H))
                            )

                    xt = dpool.tile([128, KK * T], BF16, tag="xt")
                    nc.sync.dma_start(xt[:], x_in[s])

                    pss = [
                        mmpool.tile([128, 512], F32, tag="ps", name=f"ps{q}")
                        for q in range(MM * NN)
                    ]
                    for kk in range(KK):
                        wsrc = wts[kk // KPC]
                        koff = (kk % KPC) * H
                        for m in range(MM):
                            lhsT = xt[:, kk * T + m * 128 : kk * T + (m + 1) * 128]
                            for n in range(NN):
                                nc.tensor.matmul(
                                    pss[m * NN + n][:],
                                    lhsT,
                                    wsrc[:, koff + n * 512 : koff + (n + 1) * 512],
                                    start=(kk == 0),
                                    stop=(kk == KK - 1),
                                )

                    if V2:
                        ot2 = dpool.tile([128, MM * H], out_dt, tag="ot2", name="ot2")
                        ots = [ot2[:, m * H : (m + 1) * H] for m in range(MM)]
                    else:
                        ots = [
                            dpool.tile([128, H], out_dt, tag=f"ot{m}", name=f"ot{m}")
                            for m in range(MM)
                        ]
                    for m in range(MM):
                        for n in range(NN):
                            if has_bias:
                                nc.vector.tensor_add(
                                    ots[m][:, n * 512 : (n + 1) * 512],
                                    pss[m * NN + n][:],
                                    bb[:, n * 512 : (n + 1) * 512],
                                )
                            else:
                                nc.vector.tensor_copy(
                                    ots[m][:, n * 512 : (n + 1) * 512],
                                    pss[m * NN + n][:],
                                )
                    if V2:
                        nc.gpsimd.dma_start(
                            out_o[s].rearrange("(mm t) h -> t mm h", mm=MM),
                            ot2[:].rearrange("p (mm h) -> p mm h", h=H),
                        )
                    else:
                        for m in range(MM):
                            nc.sync.dma_start(
                                out_o[s, m * 128 : (m + 1) * 128, :], ots[m][:]
                            )

            if V2:
                _body()
                with tc.For_i(0, reps_val, 1, staggered_reset=True):
                    _body()
                    _body()
            else:
                with tc.For_i(0, reps_val, 1):
                    _body()

    nc.compile()
    return nc


import os as _os

DEDUPE = _os.environ.get("KERNEL_DEDUPE", "0") == "1"
V2FLAG = _os.environ.get("KERNEL_V", "1") == "2"


def _get_nc(has_bias: bool):
    key = ("nc", has_bias, DEDUPE, V2FLAG)
    if key not in _cache:
        _cache[key] = _build(has_bias, DEDUPE, V2FLAG)
    return _cache[key]


def _plan(cat_ids: np.ndarray):
    """Pack samples into NCORES bins of S slots, same-category samples
    adjacent, minimizing category-group pieces per bin. Returns per-core
    (sample index list, reload flags)."""
    groups: dict[int, list[int]] = {}
    for i, c in enumerate(cat_ids.tolist()):
        groups.setdefault(c, []).append(i)
    # largest groups first
    order = sorted(groups.items(), key=lambda kv: -len(kv[1]))
    cap = [S] * NCORES
    pieces: list[list[tuple[int, list[int]]]] = [[] for _ in range(NCORES)]
    for cat, idxs in order:
        rest = list(idxs)
        while rest:
            # bin with most remaining capacity; ties -> fewest pieces
            c = max(range(NCORES), key=lambda b: (cap[b], -len(pieces[b])))
            take = min(len(rest), cap[c])
            assert take > 0
            pieces[c].append((cat, rest[:take]))
            cap[c] -= take
            rest = rest[take:]
    idx = np.empty((NCORES, S), dtype=np.int64)
    flags = np.zeros((NCORES, S), dtype=np.int32)
    cats = np.empty((NCORES, S), dtype=np.int32)
    for c in range(NCORES):
        s = 0
        for cat, idxs in pieces[c]:
            flags[c, s] = 1
            for i in idxs:
                idx[c, s] = i
                cats[c, s] = cat
                s += 1
        assert s == S
    return idx, flags, cats


def _make_in_maps(x, cat_ids, W, b, has_bias, plan, reps=1):
    idx, flags, cats = plan
    Wb = np.ascontiguousarray(
        W.reshape(NCAT, KK, 128, H).astype(NPBF16, copy=False)
    )
    in_maps = []
    for c in range(NCORES):
        xs = (
            x[idx[c]]
            .transpose(0, 2, 1)
            .reshape(S, KK, 128, T)
            .transpose(0, 2, 1, 3)
            .reshape(S, 128, KK * T)
            .astype(NPBF16)
        )
        Ws = np.zeros((S, KK, 128, H), dtype=NPBF16)
        for s in range(S):
            if flags[c, s] or not DEDUPE:
                Ws[s] = Wb[cats[c, s]]
        if V2FLAG:
            assert reps >= 1 and reps % 2 == 1, f"V2 needs odd reps, got {reps}"
            rv = (reps - 1) // 2
        else:
            rv = reps
        m = {
            "xs": np.ascontiguousarray(xs),
            "Ws": Ws,
            "flags": flags[c].reshape(1, S),
            "reps": np.full((1, 1), rv, dtype=np.int32),
        }
        if has_bias:
            m["bs"] = np.ascontiguousarray(b[cats[c]].astype(np.float32))
        in_maps.append(m)
    return in_maps


def kernel(x, cat_ids, W, b):
    x = np.ascontiguousarray(np.asarray(x, dtype=np.float32))
    cat_ids = np.asarray(cat_ids, dtype=np.int32)
    W = np.ascontiguousarray(np.asarray(W, dtype=np.float32))
    b = np.ascontiguousarray(np.asarray(b, dtype=np.float32))
    assert x.shape == (B, T, I) and cat_ids.shape == (B,)
    assert W.shape == (NCAT, I, H) and b.shape == (NCAT, H)

    has_bias = bool(np.any(b))
    nc = _get_nc(has_bias)
    plan = _plan(cat_ids)
    in_maps = _make_in_maps(x, cat_ids, W, b, has_bias, plan)

    res = run_bass_kernel_spmd(nc, in_maps, list(range(NCORES)))

    idx = plan[0]
    out = np.empty((B, T, H), dtype=np.float32)
    for c in range(NCORES):
        out[idx[c]] = np.asarray(res.results[c]["out"]).astype(np.float32)
    return out
